# revision 5
# baseline (speedup 1.0000x reference)
"""CoarseMatching (retrieval_knn) Trainium2 kernel.

Reference computation:
    d2[n,m]  = ||r_n||^2 + ||s_m||^2 - 2 r_n.s_m          (N=M=8192, D=256)
    S        = exp(-d2)
    F        = (S / (rowsum(S)+1e-8)) * (S / (colsum(S)+1e-8))
    top-k of F over all (n,m), k = num_correspondences (256)

Strategy (8 NeuronCores, ref rows sharded 1024/core, single pass):
  Per [128n, 512m] tile: PE computes H = 2*P - ns[m] via fp32r matmuls
  (2x K=128 + 1x K=1 bias row), ScalarE computes S = exp(H - nr[n]) with
  fused row-sum accumulation, PE accumulates column sums with a ones-
  matmul over the fp32r-rounded S, VectorE takes the per-block max of H.
  Devices emit only row-sum partials, column-sum partials and block maxima
  (no on-device top-k). The host then:
    1. assembles exact-enough R[n], C[m] (correcting the systematic
       column perturbation from rounding ns to fp32r's e8m11 format),
    2. ranks blocks by a provable upper bound on G/2 = log(F)/2 and runs
       an exact lazy top-k: rescore candidate blocks in f64 from the
       ORIGINAL f32 inputs until the k-th best exact value dominates all
       remaining block bounds,
    3. orders the k winners like jax.lax.top_k (value desc, index asc on
       f32 ties) and computes their F scores in f64.

fp32r note: TRN2's full-rate fp32 matmul mode requires operands rounded to
e8m11 (top 20 bits of the fp32 encoding). The rounding error averages out
in the row/col sums and is absorbed by an explicit margin in the block
bounds; the final ranking is computed on the host from unrounded inputs.
"""

import numpy as np

# ---- problem constants (hardcoded per contract) ----
N = 8192
M = 8192
D = 256
NCORES = 8
RPC = N // NCORES          # 1024 ref rows per core
PT = 128                   # partition tile (rows per tile)
NT = RPC // PT             # 8 row tiles per core
BLK = 512                  # column block
NJ = M // BLK              # 16 column blocks
UB_MARGIN = 1e-3           # slack for fp32r rounding in block bounds (G/2 units)

_COMPILED = {}


def _round_f32r(x: np.ndarray) -> np.ndarray:
    """Round float32 to fp32r (e8m11: top 20 bits of the fp32 encoding), RNE."""
    u = np.ascontiguousarray(x, np.float32).view(np.uint32).astype(np.uint64)
    lsb = (u >> np.uint64(12)) & np.uint64(1)
    r = (u + np.uint64(0x7FF) + lsb) & np.uint64(0xFFFFF000)
    return r.astype(np.uint32).view(np.float32)


def _build_program():
    from contextlib import ExitStack

    import concourse.mybir as mybir
    import concourse.tile as tile
    from concourse import bacc

    F32 = mybir.dt.float32
    F32R = mybir.dt.float32r

    nc = bacc.Bacc("TRN2", target_bir_lowering=False, debug=False,
                   num_devices=NCORES)

    refT = nc.dram_tensor("refT", [D, RPC], F32R, kind="ExternalInput").ap()
    srcT = nc.dram_tensor("srcT", [D, M], F32R, kind="ExternalInput").ap()
    nsrow = nc.dram_tensor("nsrow", [1, M], F32R, kind="ExternalInput").ap()
    nrneg = nc.dram_tensor("nrneg", [PT, NT], F32, kind="ExternalInput").ap()
    onesk1 = nc.dram_tensor("onesk1", [1, PT], F32R, kind="ExternalInput").ap()
    onescol = nc.dram_tensor("onescol", [PT, 1], F32R, kind="ExternalInput").ap()

    rpart = nc.dram_tensor("rpart", [PT, NT * NJ], F32, kind="ExternalOutput").ap()
    bmout = nc.dram_tensor("bmout", [PT, NT * NJ], F32, kind="ExternalOutput").ap()
    csub = nc.dram_tensor("csub", [1, M], F32, kind="ExternalOutput").ap()

    with tile.TileContext(nc) as tc, ExitStack() as ctx:
        sb = ctx.enter_context(tc.tile_pool(name="sb", bufs=1))
        spool = ctx.enter_context(tc.tile_pool(name="spool", bufs=3))
        ps_h = ctx.enter_context(tc.tile_pool(name="ps_h", bufs=3, space="PSUM"))
        ps_c = ctx.enter_context(tc.tile_pool(name="ps_c", bufs=2, space="PSUM"))

        rT = [sb.tile([PT, RPC], F32R, tag=f"rT{h}", name=f"rT{h}")
              for h in range(2)]
        sT = [sb.tile([PT, M], F32R, tag=f"sT{h}", name=f"sT{h}")
              for h in range(2)]
        nst = sb.tile([1, M], F32R, tag="nst")
        nrt = sb.tile([PT, NT], F32, tag="nrt")
        o1 = sb.tile([1, PT], F32R, tag="o1")
        oc = sb.tile([PT, 1], F32R, tag="oc")
        rp = sb.tile([PT, NT * NJ], F32, tag="rp")
        bm = sb.tile([PT, NT * NJ], F32, tag="bm")
        cs = sb.tile([1, M], F32, tag="cs")

        nc.sync.dma_start(out=nst[:], in_=nsrow[:, :])
        nc.sync.dma_start(out=nrt[:], in_=nrneg[:, :])
        nc.sync.dma_start(out=o1[:], in_=onesk1[:, :])
        nc.sync.dma_start(out=oc[:], in_=onescol[:, :])
        for h in range(2):
            nc.sync.dma_start(out=rT[h][:], in_=refT[h * PT:(h + 1) * PT, :])
        for j in range(NJ):
            cols = slice(j * BLK, (j + 1) * BLK)
            for h in range(2):
                nc.sync.dma_start(out=sT[h][:, cols],
                                  in_=srcT[h * PT:(h + 1) * PT, cols])

        pending = None   # (s_tile, cps, start, stop)
        done_cps = None  # (cps, j) whose colsum finished and needs copy-out

        for j in range(NJ):
            cols = slice(j * BLK, (j + 1) * BLK)
            cps = ps_c.tile([1, BLK], F32, tag="cps")
            for t in range(NT):
                rows = slice(t * PT, (t + 1) * PT)
                kk = t * NJ + j
                h_ps = ps_h.tile([PT, BLK], F32, tag="h")
                nc.tensor.matmul(h_ps[:], rT[0][:, rows], sT[0][:, cols],
                                 start=True, stop=False)
                nc.tensor.matmul(h_ps[:], rT[1][:, rows], sT[1][:, cols],
                                 start=False, stop=False)
                nc.tensor.matmul(h_ps[:], o1[:], nst[:, cols],
                                 start=False, stop=True)
                if pending is not None:
                    ts_, cps_, st_, sp_ = pending
                    nc.tensor.matmul(cps_[:], oc[:], ts_[:], start=st_, stop=sp_)
                    pending = None
                    if sp_:
                        done_cps = (cps_, j - 1)
                if done_cps is not None:
                    cps_, jd = done_cps
                    nc.scalar.copy(cs[:, jd * BLK:(jd + 1) * BLK], cps_[:])
                    done_cps = None
                s_t = spool.tile([PT, BLK], F32R, tag="s")
                nc.scalar.activation(s_t[:], h_ps[:],
                                     mybir.ActivationFunctionType.Exp,
                                     bias=nrt[:, t:t + 1], scale=1.0,
                                     accum_out=rp[:, kk:kk + 1])
                nc.vector.reduce_max(bm[:, kk:kk + 1], h_ps[:],
                                     axis=mybir.AxisListType.X)
                pending = (s_t, cps, t == 0, t == NT - 1)

        ts_, cps_, st_, sp_ = pending
        nc.tensor.matmul(cps_[:], oc[:], ts_[:], start=st_, stop=sp_)
        nc.scalar.copy(cs[:, (NJ - 1) * BLK:NJ * BLK], cps_[:])

        nc.sync.dma_start(out=rpart[:, :], in_=rp[:])
        nc.sync.dma_start(out=bmout[:, :], in_=bm[:])
        nc.sync.dma_start(out=csub[:, :], in_=cs[:])

    nc.compile()
    return nc


def _get_program():
    if "nc" not in _COMPILED:
        _COMPILED["nc"] = _build_program()
    return _COMPILED["nc"]


def _run_device(ref: np.ndarray, src: np.ndarray, trace: bool = False):
    from concourse.bass_utils import run_bass_kernel_spmd

    nc = _get_program()

    nr64 = (ref.astype(np.float64) ** 2).sum(1)            # [N]
    ns64 = (src.astype(np.float64) ** 2).sum(1)            # [M]
    srcT_r = _round_f32r(np.ascontiguousarray(src.T))      # [D, M]
    ns_r = _round_f32r((-ns64).astype(np.float32).reshape(1, M))
    onesk1 = np.ones((1, PT), np.float32)
    onescol = np.ones((PT, 1), np.float32)

    in_maps = []
    for c in range(NCORES):
        rows = slice(c * RPC, (c + 1) * RPC)
        # factor 2 from d2 = nr + ns - 2P folded into the ref operand
        refT_r = _round_f32r(np.ascontiguousarray(2.0 * ref[rows].T))  # [D, RPC]
        nrneg = np.ascontiguousarray(
            (-nr64[rows]).astype(np.float32).reshape(NT, PT).T)  # [PT, NT]
        in_maps.append({
            "refT": refT_r, "srcT": srcT_r, "nsrow": ns_r, "nrneg": nrneg,
            "onesk1": onesk1, "onescol": onescol,
        })

    out = run_bass_kernel_spmd(nc, in_maps, core_ids=list(range(NCORES)),
                               trace=trace)
    res = out.results

    # assemble per-row quantities; row n = c*RPC + t*PT + p
    R = np.empty(N, np.float64)
    BM = np.empty((N, NJ), np.float64)
    Cdev = np.zeros(M, np.float64)
    for c in range(NCORES):
        rp = res[c]["rpart"].astype(np.float64)   # [PT, NT*NJ]
        bm = res[c]["bmout"].astype(np.float64)
        rp3 = rp.reshape(PT, NT, NJ)
        bm3 = bm.reshape(PT, NT, NJ)
        rows = slice(c * RPC, (c + 1) * RPC)
        R[rows] = rp3.sum(axis=2).T.reshape(RPC)
        BM[rows] = bm3.transpose(1, 0, 2).reshape(RPC, NJ)
        Cdev += res[c]["csub"].astype(np.float64).reshape(M)

    # correct the systematic per-column perturbation: device used
    # exp(ns_r) instead of exp(ns)
    ns_r64 = ns_r.reshape(M).astype(np.float64)   # this is -ns rounded
    Ccorr = Cdev * np.exp((-ns64) - ns_r64)
    return R, Ccorr, BM, nr64, ns64, ns_r64, out


def _lazy_topk(ref, src, k, R, C, BM, nr64, ns64, ns_r64):
    """Exact top-k of F.

    Device R/C (noise ~1e-5 from the exp LUT and fp32r rounding) drive only
    the block bounds and a preliminary ranking; the final ranking and scores
    use R/C recomputed exactly on the host (f64, original f32 inputs) for
    just the candidate rows/columns.
    """
    ref64 = ref.astype(np.float64)
    src64 = src.astype(np.float64)
    logR = np.log(R + 1e-8)
    logC = np.log(C + 1e-8)

    # per-block upper bound on G/2[n,m] = 2P - nr[n] - ns[m]
    #                                     - 0.5 logR[n] - 0.5 logC[m]
    # device H[n,m] = 2P_r + nsneg_r[m]  (nsneg_r = rounded -ns)
    a2 = -nr64 - 0.5 * logR                                  # [N]
    percol = (-ns64 - ns_r64) - 0.5 * logC                   # [M]
    w = percol.reshape(NJ, BLK).max(axis=1)                  # [NJ]
    UB = BM + a2[:, None] + w[None, :] + UB_MARGIN           # [N, NJ]

    flat = UB.reshape(-1)
    order = np.argsort(-flat)

    cand_val = []   # G/2 using device R/C (selection-grade)
    cand_idx = []   # flat n*M + m
    kth_best = -np.inf
    pos = 0
    CHUNK = 64
    total = flat.shape[0]
    while pos < total:
        if len(cand_val) >= CHUNK and kth_best >= flat[order[pos]]:
            break
        take = order[pos:pos + CHUNK]
        pos += CHUNK
        for b in take:
            n = int(b // NJ)
            j = int(b % NJ)
            m0 = j * BLK
            p = src64[m0:m0 + BLK] @ ref64[n]                # [BLK]
            g2 = (2.0 * p - nr64[n] - ns64[m0:m0 + BLK]
                  - 0.5 * logR[n] - 0.5 * logC[m0:m0 + BLK])
            cand_val.append(g2)
            cand_idx.append(n * M + m0 + np.arange(BLK, dtype=np.int64))
        vals = np.concatenate(cand_val)
        if vals.shape[0] >= k:
            kth_best = np.partition(vals, -k)[-k]

    vals = np.concatenate(cand_val)
    idxs = np.concatenate(cand_idx)

    # preliminary candidate set with cushion for device-R/C noise
    CUSHION = 3e-3
    kth = np.partition(vals, -k)[-k]
    keep = vals >= kth - CUSHION
    cvals = vals[keep]
    cidx = idxs[keep]
    rows = cidx // M
    cols = cidx % M

    # exact R/C (f64 from original f32 inputs) for candidate rows/cols
    urows = np.unique(rows)
    ucols = np.unique(cols)
    Pr = ref64[urows] @ src64.T                              # [ur, M]
    d2r = nr64[urows][:, None] + ns64[None, :] - 2.0 * Pr
    Rex_map = np.exp(-d2r).sum(1)                            # exact R
    Pc = src64[ucols] @ ref64.T                              # [uc, N]
    d2c = ns64[ucols][:, None] + nr64[None, :] - 2.0 * Pc
    Cex_map = np.exp(-d2c).sum(1)                            # exact C
    Rex = np.empty(N); Rex[urows] = Rex_map
    Cex = np.empty(M); Cex[ucols] = Cex_map

    # exact scores for candidates
    d2 = nr64[rows] + ns64[cols] - 2.0 * np.einsum(
        "ij,ij->i", ref64[rows], src64[cols])
    S = np.exp(-d2)
    F = (S / (Rex[rows] + 1e-8)) * (S / (Cex[cols] + 1e-8))

    top = np.argpartition(F, -k)[-k:]
    tidx = cidx[top]
    F32v = F[top].astype(np.float32)
    # mirror lax.top_k ordering: f32 value desc, flat index asc on ties
    ordr = np.lexsort((tidx, -F32v.astype(np.float64)))
    tidx = tidx[ordr]
    F32v = F32v[ordr]
    _ = cvals
    return tidx, F32v


def kernel(ref_feats, src_feats, num_correspondences):
    k = int(np.asarray(num_correspondences))
    ref = np.ascontiguousarray(np.asarray(ref_feats), dtype=np.float32)
    src = np.ascontiguousarray(np.asarray(src_feats), dtype=np.float32)
    assert ref.shape == (N, D) and src.shape == (M, D)

    R, C, BM, nr64, ns64, ns_r64, _ = _run_device(ref, src)
    tidx, F32v = _lazy_topk(ref, src, k, R, C, BM, nr64, ns64, ns_r64)

    ref_idx = (tidx // M).astype(np.int32)
    src_idx = (tidx % M).astype(np.int32)
    return ref_idx, src_idx, F32v.astype(np.float32)


# revision 6
# speedup vs baseline: 1.0890x; 1.0890x over previous
"""CoarseMatching (retrieval_knn) Trainium2 kernel.

Reference computation:
    d2[n,m]  = ||r_n||^2 + ||s_m||^2 - 2 r_n.s_m          (N=M=8192, D=256)
    S        = exp(-d2)
    F        = (S / (rowsum(S)+1e-8)) * (S / (colsum(S)+1e-8))
    top-k of F over all (n,m), k = num_correspondences (256)

Strategy (8 NeuronCores, ref rows sharded 1024/core, single pass):
  Columns are pre-permuted by ascending ||s_m||^2 so each 512-wide block
  spans a narrow range of source norms (tight block bounds below).
  Per [128n, 512m] tile on each core:
    PE    : T-metric matmul  H = 2*P  (2 fp32r matmuls, K=128 each)
            + a ones-matmul accumulating column sums of T over row tiles
    ScalarE: T = exp(H - nr[n])  (per-partition bias) with fused row-sum
            accumulation (block partials, fp32)
    VectorE: per-block max of T (monotone in the ranking metric per row)
  Devices emit only row-sum partials [128, 8x16], block maxima and column
  sums - no on-device top-k, no collectives. The host then:
    1. recovers C[m] = exp(-ns[m]) * colsum(T) exactly, estimates R[n]
       from the block partials weighted by per-block mean of exp(-ns),
    2. ranks blocks by a provable upper bound on G/2 = log(F)/2 and runs
       a lazy scan: rescore candidate blocks in f64 from the ORIGINAL f32
       inputs until the k-th best value dominates all remaining bounds,
    3. recomputes R/C exactly (f64) for just the candidate rows/columns,
       re-ranks, orders like jax.lax.top_k (f32 value desc, index asc on
       ties) and emits exact-to-f32 scores.
  All approximation error (fp32r's e8m11 operand rounding, exp-LUT noise,
  the R-hat estimate) is confined to the candidate-selection bounds and
  covered by an explicit margin; the returned indices/scores are exact.

fp32r note: TRN2's full-rate fp32 matmul mode requires operands rounded
to e8m11 (top 20 bits of the fp32 encoding, RNE) - done on the host.
"""

import numpy as np

# ---- problem constants (hardcoded per contract) ----
N = 8192
M = 8192
D = 256
NCORES = 8
RPC = N // NCORES          # 1024 ref rows per core
PT = 128                   # partition tile (rows per tile)
NT = RPC // PT             # 8 row tiles per core
BLK = 512                  # column block
NJ = M // BLK              # 16 column blocks
UB_MARGIN = 6e-3           # covers fp32r rounding + exp LUT + R-hat noise
CUSHION = 5e-3             # preliminary-ranking cushion before exact R/C

_COMPILED = {}


def _round_f32r(x: np.ndarray) -> np.ndarray:
    """Round float32 to fp32r (e8m11: top 20 bits of the fp32 encoding), RNE."""
    u = np.ascontiguousarray(x, np.float32).view(np.uint32).astype(np.uint64)
    lsb = (u >> np.uint64(12)) & np.uint64(1)
    r = (u + np.uint64(0x7FF) + lsb) & np.uint64(0xFFFFF000)
    return r.astype(np.uint32).view(np.float32)


def _build_program():
    from contextlib import ExitStack

    import concourse.mybir as mybir
    import concourse.tile as tile
    from concourse import bacc

    F32 = mybir.dt.float32
    F32R = mybir.dt.float32r

    nc = bacc.Bacc("TRN2", target_bir_lowering=False, debug=False,
                   num_devices=NCORES)

    refT = nc.dram_tensor("refT", [D, RPC], F32R, kind="ExternalInput").ap()
    srcT = nc.dram_tensor("srcT", [D, M], F32R, kind="ExternalInput").ap()
    nrneg = nc.dram_tensor("nrneg", [PT, NT], F32, kind="ExternalInput").ap()
    onescol = nc.dram_tensor("onescol", [PT, 1], F32R, kind="ExternalInput").ap()

    rpart = nc.dram_tensor("rpart", [PT, NT * NJ], F32, kind="ExternalOutput").ap()
    bmout = nc.dram_tensor("bmout", [PT, NT * NJ], F32, kind="ExternalOutput").ap()
    csub = nc.dram_tensor("csub", [1, M], F32, kind="ExternalOutput").ap()

    with tile.TileContext(nc) as tc, ExitStack() as ctx:
        sb = ctx.enter_context(tc.tile_pool(name="sb", bufs=1))
        spool = ctx.enter_context(tc.tile_pool(name="spool", bufs=3))
        ps_h = ctx.enter_context(tc.tile_pool(name="ps_h", bufs=3, space="PSUM"))
        ps_c = ctx.enter_context(tc.tile_pool(name="ps_c", bufs=2, space="PSUM"))

        rT = [sb.tile([PT, RPC], F32R, tag=f"rT{h}", name=f"rT{h}")
              for h in range(2)]
        sT = [sb.tile([PT, M], F32R, tag=f"sT{h}", name=f"sT{h}")
              for h in range(2)]
        nrt = sb.tile([PT, NT], F32, tag="nrt")
        oc = sb.tile([PT, 1], F32R, tag="oc")
        rp = sb.tile([PT, NT * NJ], F32, tag="rp")
        bm = sb.tile([PT, NT * NJ], F32, tag="bm")
        cs = sb.tile([1, M], F32, tag="cs")

        nc.sync.dma_start(out=nrt[:], in_=nrneg[:, :])
        nc.sync.dma_start(out=oc[:], in_=onescol[:, :])
        for h in range(2):
            nc.sync.dma_start(out=rT[h][:], in_=refT[h * PT:(h + 1) * PT, :])
        for j in range(NJ):
            cols = slice(j * BLK, (j + 1) * BLK)
            for h in range(2):
                nc.sync.dma_start(out=sT[h][:, cols],
                                  in_=srcT[h * PT:(h + 1) * PT, cols])

        pending = None   # (s_tile, cps, start, stop)
        done_cps = None  # (cps, j) whose colsum finished and needs copy-out

        for j in range(NJ):
            cols = slice(j * BLK, (j + 1) * BLK)
            cps = ps_c.tile([1, BLK], F32, tag="cps")
            for t in range(NT):
                rows = slice(t * PT, (t + 1) * PT)
                kk = t * NJ + j
                h_ps = ps_h.tile([PT, BLK], F32, tag="h")
                nc.tensor.matmul(h_ps[:], rT[0][:, rows], sT[0][:, cols],
                                 start=True, stop=False)
                nc.tensor.matmul(h_ps[:], rT[1][:, rows], sT[1][:, cols],
                                 start=False, stop=True)
                # previous tile's colsum matmul: PE never waits on this
                # iteration's ScalarE output
                if pending is not None:
                    ts_, cps_, st_, sp_ = pending
                    nc.tensor.matmul(cps_[:], oc[:], ts_[:], start=st_, stop=sp_)
                    pending = None
                    if sp_:
                        done_cps = (cps_, j - 1)
                if done_cps is not None:
                    cps_, jd = done_cps
                    nc.scalar.copy(cs[:, jd * BLK:(jd + 1) * BLK], cps_[:])
                    done_cps = None
                s_t = spool.tile([PT, BLK], F32R, tag="s")
                nc.scalar.activation(s_t[:], h_ps[:],
                                     mybir.ActivationFunctionType.Exp,
                                     bias=nrt[:, t:t + 1], scale=1.0,
                                     accum_out=rp[:, kk:kk + 1])
                nc.vector.reduce_max(bm[:, kk:kk + 1], s_t[:].bitcast(F32),
                                     axis=mybir.AxisListType.X)
                pending = (s_t, cps, t == 0, t == NT - 1)

        ts_, cps_, st_, sp_ = pending
        nc.tensor.matmul(cps_[:], oc[:], ts_[:], start=st_, stop=sp_)
        nc.scalar.copy(cs[:, (NJ - 1) * BLK:NJ * BLK], cps_[:])

        nc.sync.dma_start(out=rpart[:, :], in_=rp[:])
        nc.sync.dma_start(out=bmout[:, :], in_=bm[:])
        nc.sync.dma_start(out=csub[:, :], in_=cs[:])

    nc.compile()
    return nc


def _get_program():
    if "nc" not in _COMPILED:
        _COMPILED["nc"] = _build_program()
    return _COMPILED["nc"]


def _run_device(ref: np.ndarray, src: np.ndarray, perm: np.ndarray,
                nr64: np.ndarray, trace: bool = False):
    """Run the SPMD program. `perm` is the ns-ascending column permutation;
    srcT is uploaded in permuted order and all per-column outputs are in
    permuted space."""
    from concourse.bass_utils import run_bass_kernel_spmd

    nc = _get_program()

    src_p = src[perm]                                       # [M, D] permuted
    srcT_r = _round_f32r(np.ascontiguousarray(src_p.T))     # [D, M]
    onescol = np.ones((PT, 1), np.float32)

    in_maps = []
    for c in range(NCORES):
        rows = slice(c * RPC, (c + 1) * RPC)
        # factor 2 from d2 = nr + ns - 2P folded into the ref operand
        refT_r = _round_f32r(np.ascontiguousarray(2.0 * ref[rows].T))
        nrneg = np.ascontiguousarray(
            (-nr64[rows]).astype(np.float32).reshape(NT, PT).T)  # [PT, NT]
        in_maps.append({
            "refT": refT_r, "srcT": srcT_r, "nrneg": nrneg,
            "onescol": onescol,
        })

    out = run_bass_kernel_spmd(nc, in_maps, core_ids=list(range(NCORES)),
                               trace=trace)
    res = out.results

    # assemble; row n = c*RPC + t*PT + p, columns in permuted space
    RP = np.empty((N, NJ), np.float64)   # block partials of rowsum(T)
    BM = np.empty((N, NJ), np.float64)   # block max of T (fp32r-rounded)
    B = np.zeros(M, np.float64)          # colsum of T, permuted
    for c in range(NCORES):
        rp = res[c]["rpart"].astype(np.float64).reshape(PT, NT, NJ)
        bmv = res[c]["bmout"].astype(np.float64).reshape(PT, NT, NJ)
        rows = slice(c * RPC, (c + 1) * RPC)
        RP[rows] = rp.transpose(1, 0, 2).reshape(RPC, NJ)
        BM[rows] = bmv.transpose(1, 0, 2).reshape(RPC, NJ)
        B += res[c]["csub"].astype(np.float64).reshape(M)
    return RP, BM, B, out


def _host_topk(ref, src, k, perm, nr64, ns64, RP, BM, B):
    """Exact top-k of F from device summaries.

    T[n,m'] = exp(2 P - nr[n]) over permuted columns m'. Device gives
    row-block partial sums RP, block maxima BM (of fp32r-rounded T) and
    column sums B. Selection uses bounds with margin; final ranking and
    scores use exact f64 R/C for candidate rows/cols only.
    """
    ref64 = ref.astype(np.float64)
    src64 = src.astype(np.float64)
    ns_p = ns64[perm]                                        # permuted ns
    es_p = np.exp(-ns_p)

    C_p = es_p * B                                           # exact-grade C
    esbar = es_p.reshape(NJ, BLK).mean(axis=1)               # [NJ]
    Rhat = RP @ esbar                                        # [N] estimate
    logR = np.log(Rhat + 1e-8)
    logC_p = np.log(C_p + 1e-8)

    # G/2[n,m'] = 2P - nr[n] - ns[m'] - 0.5 logR[n] - 0.5 logC[m']
    #   max(2P) over block = log(BM) + nr[n]   (nr cancels in the bound)
    # UB[n,j] = log(BM[n,j]) - 0.5 logR[n] + max_{m' in j}(-ns - 0.5 logC)
    percol = -ns_p - 0.5 * logC_p                            # [M] permuted
    w = percol.reshape(NJ, BLK).max(axis=1)                  # [NJ]
    UB = np.log(np.maximum(BM, 1e-300)) - 0.5 * logR[:, None] \
        + w[None, :] + UB_MARGIN

    flat = UB.reshape(-1)
    order = np.argsort(-flat)

    cand_val = []   # G/2 using device-derived R-hat/C (selection-grade)
    cand_idx = []   # flat n*M + m (TRUE column ids)
    kth_best = -np.inf
    pos = 0
    CHUNK = 64
    total = flat.shape[0]
    while pos < total:
        if len(cand_val) * BLK >= k and kth_best >= flat[order[pos]]:
            break
        take = order[pos:pos + CHUNK]
        pos += CHUNK
        for b in take:
            n = int(b // NJ)
            j = int(b % NJ)
            mcols = perm[j * BLK:(j + 1) * BLK]              # true col ids
            p = src64[mcols] @ ref64[n]                      # [BLK]
            g2 = (2.0 * p - nr64[n] - ns64[mcols]
                  - 0.5 * logR[n] - 0.5 * logC_p[j * BLK:(j + 1) * BLK])
            cand_val.append(g2)
            cand_idx.append(n * M + mcols.astype(np.int64))
        vals = np.concatenate(cand_val)
        if vals.shape[0] >= k:
            kth_best = np.partition(vals, -k)[-k]

    vals = np.concatenate(cand_val)
    idxs = np.concatenate(cand_idx)

    kth = np.partition(vals, -k)[-k]
    keep = vals >= kth - CUSHION
    cidx = idxs[keep]
    rows = cidx // M
    cols = cidx % M

    # exact R/C (f64 from original f32 inputs) for candidate rows/cols
    urows = np.unique(rows)
    ucols = np.unique(cols)
    Pr = ref64[urows] @ src64.T
    Rex_map = np.exp(-(nr64[urows][:, None] + ns64[None, :] - 2.0 * Pr)).sum(1)
    Pc = src64[ucols] @ ref64.T
    Cex_map = np.exp(-(ns64[ucols][:, None] + nr64[None, :] - 2.0 * Pc)).sum(1)
    Rex = np.empty(N)
    Rex[urows] = Rex_map
    Cex = np.empty(M)
    Cex[ucols] = Cex_map

    d2 = nr64[rows] + ns64[cols] - 2.0 * np.einsum(
        "ij,ij->i", ref64[rows], src64[cols])
    S = np.exp(-d2)
    F = (S / (Rex[rows] + 1e-8)) * (S / (Cex[cols] + 1e-8))

    top = np.argpartition(F, -k)[-k:]
    tidx = cidx[top]
    F32v = F[top].astype(np.float32)
    # mirror lax.top_k ordering: f32 value desc, flat index asc on ties
    ordr = np.lexsort((tidx, -F32v.astype(np.float64)))
    return tidx[ordr], F32v[ordr]


def kernel(ref_feats, src_feats, num_correspondences):
    k = int(np.asarray(num_correspondences))
    ref = np.ascontiguousarray(np.asarray(ref_feats), dtype=np.float32)
    src = np.ascontiguousarray(np.asarray(src_feats), dtype=np.float32)
    assert ref.shape == (N, D) and src.shape == (M, D)

    nr64 = (ref.astype(np.float64) ** 2).sum(1)
    ns64 = (src.astype(np.float64) ** 2).sum(1)
    perm = np.argsort(ns64, kind="stable")

    RP, BM, B, _ = _run_device(ref, src, perm, nr64)
    tidx, F32v = _host_topk(ref, src, k, perm, nr64, ns64, RP, BM, B)

    ref_idx = (tidx // M).astype(np.int32)
    src_idx = (tidx % M).astype(np.int32)
    return ref_idx, src_idx, F32v.astype(np.float32)


# revision 10
# speedup vs baseline: 1.3097x; 1.2027x over previous
"""CoarseMatching (retrieval_knn) Trainium2 kernel.

Reference computation:
    d2[n,m]  = ||r_n||^2 + ||s_m||^2 - 2 r_n.s_m          (N=M=8192, D=256)
    S        = exp(-d2)
    F        = (S / (rowsum(S)+1e-8)) * (S / (colsum(S)+1e-8))
    top-k of F over all (n,m), k = num_correspondences (256)

Strategy (8 NeuronCores, ref rows sharded 1024/core, single pass):
  Columns are pre-permuted by ascending ||s_m||^2 so each 512-wide block
  spans a narrow range of source norms (tight block bounds below).
  Per [128n, 512m] tile on each core:
    PE    : T-metric matmul  H = 2*P  (2 fp32r matmuls, K=128 each)
            + a ones-matmul accumulating column sums of T over row tiles
    ScalarE: T = exp(H - nr[n])  (per-partition bias) with fused row-sum
            accumulation (block partials, fp32)
    VectorE: per-block max of T (monotone in the ranking metric per row)
  Devices emit only row-sum partials [128, 8x16], block maxima and column
  sums - no on-device top-k, no collectives. The host then:
    1. recovers C[m] = exp(-ns[m]) * colsum(T) exactly, estimates R[n]
       from the block partials weighted by per-block mean of exp(-ns),
    2. ranks blocks by a provable upper bound on G/2 = log(F)/2 and runs
       a lazy scan: rescore candidate blocks in f64 from the ORIGINAL f32
       inputs until the k-th best value dominates all remaining bounds,
    3. recomputes R/C exactly (f64) for just the candidate rows/columns,
       re-ranks, orders like jax.lax.top_k (f32 value desc, index asc on
       ties) and emits exact-to-f32 scores.
  All approximation error (fp32r's e8m11 operand rounding, exp-LUT noise,
  the R-hat estimate) is confined to the candidate-selection bounds and
  covered by an explicit margin; the returned indices/scores are exact.

fp32r note: TRN2's full-rate fp32 matmul mode requires operands rounded
to e8m11 (top 20 bits of the fp32 encoding, RNE) - done on the host.
"""

import numpy as np

# ---- problem constants (hardcoded per contract) ----
N = 8192
M = 8192
D = 256
NCORES = 8
RPC = N // NCORES          # 1024 ref rows per core
PT = 128                   # partition tile (rows per tile)
NT = RPC // PT             # 8 row tiles per core
BLK = 512                  # column block
NJ = M // BLK              # 16 column blocks
UB_MARGIN = 6e-3           # covers fp32r rounding + exp LUT + R-hat noise
CUSHION = 5e-3             # preliminary-ranking cushion before exact R/C

_COMPILED = {}


def _round_f32r(x: np.ndarray) -> np.ndarray:
    """Round float32 to fp32r (e8m11: top 20 bits of the fp32 encoding), RNE."""
    u = np.ascontiguousarray(x, np.float32).view(np.uint32).astype(np.uint64)
    lsb = (u >> np.uint64(12)) & np.uint64(1)
    r = (u + np.uint64(0x7FF) + lsb) & np.uint64(0xFFFFF000)
    return r.astype(np.uint32).view(np.float32)


def _build_program():
    from contextlib import ExitStack

    import concourse.mybir as mybir
    import concourse.tile as tile
    from concourse import bacc

    F32 = mybir.dt.float32
    F32R = mybir.dt.float32r
    JB = 2 * BLK              # 1024-wide ScalarE tile (2 column blocks)
    NJ2 = M // JB             # 8

    nc = bacc.Bacc("TRN2", target_bir_lowering=False, debug=False,
                   num_devices=NCORES)

    refT = nc.dram_tensor("refT", [D, RPC], F32R, kind="ExternalInput").ap()
    srcT = nc.dram_tensor("srcT", [D, M], F32R, kind="ExternalInput").ap()
    nrneg = nc.dram_tensor("nrneg", [PT, NT], F32, kind="ExternalInput").ap()
    onescol = nc.dram_tensor("onescol", [PT, 1], F32R, kind="ExternalInput").ap()

    rpart = nc.dram_tensor("rpart", [PT, NT * NJ2], F32, kind="ExternalOutput").ap()
    bmout = nc.dram_tensor("bmout", [PT, NT * NJ], F32, kind="ExternalOutput").ap()
    csub = nc.dram_tensor("csub", [1, M], F32, kind="ExternalOutput").ap()

    with tile.TileContext(nc) as tc, ExitStack() as ctx:
        sb = ctx.enter_context(tc.tile_pool(name="sb", bufs=1))
        spool = ctx.enter_context(tc.tile_pool(name="spool", bufs=3))
        ps_h = ctx.enter_context(tc.tile_pool(name="ps_h", bufs=2, space="PSUM"))
        ps_c = ctx.enter_context(tc.tile_pool(name="ps_c", bufs=2, space="PSUM"))

        rT = [sb.tile([PT, RPC], F32R, tag=f"rT{h}", name=f"rT{h}")
              for h in range(2)]
        sT = [sb.tile([PT, M], F32R, tag=f"sT{h}", name=f"sT{h}")
              for h in range(2)]
        nrt = sb.tile([PT, NT], F32, tag="nrt")
        oc = sb.tile([PT, 1], F32R, tag="oc")
        rp = sb.tile([PT, NT * NJ2], F32, tag="rp")
        bm = sb.tile([PT, NT * NJ], F32, tag="bm")
        cs = sb.tile([1, M], F32, tag="cs")

        nc.sync.dma_start(out=nrt[:], in_=nrneg[:, :])
        nc.sync.dma_start(out=oc[:], in_=onescol[:, :])
        # prologue: just enough for the first tile-pair, then stream the rest
        for h in range(2):
            nc.sync.dma_start(out=rT[h][:, 0:PT], in_=refT[h * PT:(h + 1) * PT, 0:PT])
        for h in range(2):
            nc.sync.dma_start(out=sT[h][:, 0:JB], in_=srcT[h * PT:(h + 1) * PT, 0:JB])
        for h in range(2):
            nc.gpsimd.dma_start(out=rT[h][:, PT:RPC],
                                in_=refT[h * PT:(h + 1) * PT, PT:RPC])
        for j2 in range(1, NJ2):
            cols = slice(j2 * JB, (j2 + 1) * JB)
            for h in range(2):
                nc.gpsimd.dma_start(out=sT[h][:, cols],
                                    in_=srcT[h * PT:(h + 1) * PT, cols])

        pending = None   # (s2, cps, start, stop)
        done_cps = None  # (cps, j2) finished colsum pair needing copy-out

        for j2 in range(NJ2):
            c0 = j2 * JB
            cps = [ps_c.tile([1, BLK], F32, tag=f"cps{i}", name=f"cps{i}_{j2}")
                   for i in range(2)]
            for t in range(NT):
                rows = slice(t * PT, (t + 1) * PT)
                h2 = ps_h.tile([PT, JB], F32, tag="h2")
                for half in range(2):
                    hc = slice(c0 + half * BLK, c0 + (half + 1) * BLK)
                    po = h2[:, half * BLK:(half + 1) * BLK]
                    nc.tensor.matmul(po, rT[0][:, rows], sT[0][:, hc],
                                     start=True, stop=False)
                    nc.tensor.matmul(po, rT[1][:, rows], sT[1][:, hc],
                                     start=False, stop=True)
                if pending is not None:
                    s2_, cps_, st_, sp_ = pending
                    for half in range(2):
                        nc.tensor.matmul(
                            cps_[half][:], oc[:],
                            s2_[:, half * BLK:(half + 1) * BLK],
                            start=st_, stop=sp_)
                    pending = None
                    if sp_:
                        done_cps = (cps_, j2 - 1)
                if done_cps is not None:
                    cps_, jd = done_cps
                    for half in range(2):
                        nc.vector.tensor_copy(
                            cs[:, jd * JB + half * BLK:
                               jd * JB + (half + 1) * BLK],
                            cps_[half][:])
                    done_cps = None
                s2 = spool.tile([PT, JB], F32R, tag="s2")
                nc.scalar.activation(s2[:], h2[:],
                                     mybir.ActivationFunctionType.Exp,
                                     bias=nrt[:, t:t + 1], scale=1.0,
                                     accum_out=rp[:, t * NJ2 + j2:t * NJ2 + j2 + 1])
                for half in range(2):
                    kk = t * NJ + 2 * j2 + half
                    nc.vector.reduce_max(
                        bm[:, kk:kk + 1],
                        s2[:, half * BLK:(half + 1) * BLK].bitcast(F32),
                        axis=mybir.AxisListType.X)
                pending = (s2, cps, t == 0, t == NT - 1)

        s2_, cps_, st_, sp_ = pending
        for half in range(2):
            nc.tensor.matmul(cps_[half][:], oc[:],
                             s2_[:, half * BLK:(half + 1) * BLK],
                             start=st_, stop=sp_)
            nc.vector.tensor_copy(
                cs[:, (NJ2 - 1) * JB + half * BLK:
                   (NJ2 - 1) * JB + (half + 1) * BLK],
                cps_[half][:])

        nc.sync.dma_start(out=rpart[:, :], in_=rp[:])
        nc.sync.dma_start(out=csub[:, :], in_=cs[:])
        nc.sync.dma_start(out=bmout[:, :], in_=bm[:])

    nc.compile()
    return nc


def _get_program():
    if "nc" not in _COMPILED:
        _COMPILED["nc"] = _build_program()
    return _COMPILED["nc"]


def _run_device(ref: np.ndarray, src: np.ndarray, perm: np.ndarray,
                nr64: np.ndarray, trace: bool = False):
    """Run the SPMD program. `perm` is the ns-ascending column permutation;
    srcT is uploaded in permuted order and all per-column outputs are in
    permuted space."""
    from concourse.bass_utils import run_bass_kernel_spmd

    nc = _get_program()

    src_p = src[perm]                                       # [M, D] permuted
    srcT_r = _round_f32r(np.ascontiguousarray(src_p.T))     # [D, M]
    onescol = np.ones((PT, 1), np.float32)

    in_maps = []
    for c in range(NCORES):
        rows = slice(c * RPC, (c + 1) * RPC)
        # factor 2 from d2 = nr + ns - 2P folded into the ref operand
        refT_r = _round_f32r(np.ascontiguousarray(2.0 * ref[rows].T))
        nrneg = np.ascontiguousarray(
            (-nr64[rows]).astype(np.float32).reshape(NT, PT).T)  # [PT, NT]
        in_maps.append({
            "refT": refT_r, "srcT": srcT_r, "nrneg": nrneg,
            "onescol": onescol,
        })

    out = run_bass_kernel_spmd(nc, in_maps, core_ids=list(range(NCORES)),
                               trace=trace)
    res = out.results

    # assemble; row n = c*RPC + t*PT + p, columns in permuted space
    NJ2 = NJ // 2
    RP = np.empty((N, NJ2), np.float64)  # 1024-wide block partials
    BM = np.empty((N, NJ), np.float64)   # block max of T (fp32r-rounded)
    B = np.zeros(M, np.float64)          # colsum of T, permuted
    for c in range(NCORES):
        rp = res[c]["rpart"].astype(np.float64).reshape(PT, NT, NJ2)
        bmv = res[c]["bmout"].astype(np.float64).reshape(PT, NT, NJ)
        rows = slice(c * RPC, (c + 1) * RPC)
        RP[rows] = rp.transpose(1, 0, 2).reshape(RPC, NJ2)
        BM[rows] = bmv.transpose(1, 0, 2).reshape(RPC, NJ)
        B += res[c]["csub"].astype(np.float64).reshape(M)
    return RP, BM, B, out


def _host_topk(ref, src, k, perm, nr64, ns64, RP, BM, B):
    """Exact top-k of F from device summaries.

    T[n,m'] = exp(2 P - nr[n]) over permuted columns m'. Device gives
    row-block partial sums RP, block maxima BM (of fp32r-rounded T) and
    column sums B. Selection uses bounds with margin; final ranking and
    scores use exact f64 R/C for candidate rows/cols only.
    """
    ref64 = ref.astype(np.float64)
    src64 = src.astype(np.float64)
    ns_p = ns64[perm]                                        # permuted ns
    es_p = np.exp(-ns_p)

    C_p = es_p * B                                           # exact-grade C
    esbar = es_p.reshape(NJ // 2, 2 * BLK).mean(axis=1)      # [NJ//2]
    Rhat = RP @ esbar                                        # [N] estimate
    logR = np.log(Rhat + 1e-8)
    logC_p = np.log(C_p + 1e-8)

    # G/2[n,m'] = 2P - nr[n] - ns[m'] - 0.5 logR[n] - 0.5 logC[m']
    #   max(2P) over block = log(BM) + nr[n]   (nr cancels in the bound)
    # UB[n,j] = log(BM[n,j]) - 0.5 logR[n] + max_{m' in j}(-ns - 0.5 logC)
    percol = -ns_p - 0.5 * logC_p                            # [M] permuted
    w = percol.reshape(NJ, BLK).max(axis=1)                  # [NJ]
    UB = np.log(np.maximum(BM, 1e-300)) - 0.5 * logR[:, None] \
        + w[None, :] + UB_MARGIN

    flat = UB.reshape(-1)
    order = np.argsort(-flat)

    cand_val = []   # G/2 using device-derived R-hat/C (selection-grade)
    cand_idx = []   # flat n*M + m (TRUE column ids)
    kth_best = -np.inf
    pos = 0
    CHUNK = 64
    total = flat.shape[0]
    while pos < total:
        if len(cand_val) * BLK >= k and kth_best >= flat[order[pos]]:
            break
        take = order[pos:pos + CHUNK]
        pos += CHUNK
        for b in take:
            n = int(b // NJ)
            j = int(b % NJ)
            mcols = perm[j * BLK:(j + 1) * BLK]              # true col ids
            p = src64[mcols] @ ref64[n]                      # [BLK]
            g2 = (2.0 * p - nr64[n] - ns64[mcols]
                  - 0.5 * logR[n] - 0.5 * logC_p[j * BLK:(j + 1) * BLK])
            cand_val.append(g2)
            cand_idx.append(n * M + mcols.astype(np.int64))
        vals = np.concatenate(cand_val)
        if vals.shape[0] >= k:
            kth_best = np.partition(vals, -k)[-k]

    vals = np.concatenate(cand_val)
    idxs = np.concatenate(cand_idx)

    kth = np.partition(vals, -k)[-k]
    keep = vals >= kth - CUSHION
    cidx = idxs[keep]
    rows = cidx // M
    cols = cidx % M

    # exact R/C (f64 from original f32 inputs) for candidate rows/cols
    urows = np.unique(rows)
    ucols = np.unique(cols)
    Pr = ref64[urows] @ src64.T
    Rex_map = np.exp(-(nr64[urows][:, None] + ns64[None, :] - 2.0 * Pr)).sum(1)
    Pc = src64[ucols] @ ref64.T
    Cex_map = np.exp(-(ns64[ucols][:, None] + nr64[None, :] - 2.0 * Pc)).sum(1)
    Rex = np.empty(N)
    Rex[urows] = Rex_map
    Cex = np.empty(M)
    Cex[ucols] = Cex_map

    d2 = nr64[rows] + ns64[cols] - 2.0 * np.einsum(
        "ij,ij->i", ref64[rows], src64[cols])
    S = np.exp(-d2)
    F = (S / (Rex[rows] + 1e-8)) * (S / (Cex[cols] + 1e-8))

    top = np.argpartition(F, -k)[-k:]
    tidx = cidx[top]
    F32v = F[top].astype(np.float32)
    # mirror lax.top_k ordering: f32 value desc, flat index asc on ties
    ordr = np.lexsort((tidx, -F32v.astype(np.float64)))
    return tidx[ordr], F32v[ordr]


def kernel(ref_feats, src_feats, num_correspondences):
    k = int(np.asarray(num_correspondences))
    ref = np.ascontiguousarray(np.asarray(ref_feats), dtype=np.float32)
    src = np.ascontiguousarray(np.asarray(src_feats), dtype=np.float32)
    assert ref.shape == (N, D) and src.shape == (M, D)

    nr64 = (ref.astype(np.float64) ** 2).sum(1)
    ns64 = (src.astype(np.float64) ** 2).sum(1)
    perm = np.argsort(ns64, kind="stable")

    RP, BM, B, _ = _run_device(ref, src, perm, nr64)
    tidx, F32v = _host_topk(ref, src, k, perm, nr64, ns64, RP, BM, B)

    ref_idx = (tidx // M).astype(np.int32)
    src_idx = (tidx % M).astype(np.int32)
    return ref_idx, src_idx, F32v.astype(np.float32)


# revision 11
# speedup vs baseline: 1.3141x; 1.0033x over previous
"""CoarseMatching (retrieval_knn) Trainium2 kernel.

Reference computation:
    d2[n,m]  = ||r_n||^2 + ||s_m||^2 - 2 r_n.s_m          (N=M=8192, D=256)
    S        = exp(-d2)
    F        = (S / (rowsum(S)+1e-8)) * (S / (colsum(S)+1e-8))
    top-k of F over all (n,m), k = num_correspondences (256)

Strategy (8 NeuronCores, ref rows sharded 1024/core, single pass):
  Columns are pre-permuted by ascending ||s_m||^2 so each 512-wide block
  spans a narrow range of source norms (tight block bounds below).
  Per [128n, 512m] tile on each core:
    PE    : T-metric matmul  H = 2*P  (2 fp32r matmuls, K=128 each)
            + a ones-matmul accumulating column sums of T over row tiles
    ScalarE: T = exp(H - nr[n])  (per-partition bias) with fused row-sum
            accumulation (block partials, fp32)
    VectorE: per-block max of T (monotone in the ranking metric per row)
  Devices emit only row-sum partials [128, 8x16], block maxima and column
  sums - no on-device top-k, no collectives. The host then:
    1. recovers C[m] = exp(-ns[m]) * colsum(T) exactly, estimates R[n]
       from the block partials weighted by per-block mean of exp(-ns),
    2. ranks blocks by a provable upper bound on G/2 = log(F)/2 and runs
       a lazy scan: rescore candidate blocks in f64 from the ORIGINAL f32
       inputs until the k-th best value dominates all remaining bounds,
    3. recomputes R/C exactly (f64) for just the candidate rows/columns,
       re-ranks, orders like jax.lax.top_k (f32 value desc, index asc on
       ties) and emits exact-to-f32 scores.
  All approximation error (fp32r's e8m11 operand rounding, exp-LUT noise,
  the R-hat estimate) is confined to the candidate-selection bounds and
  covered by an explicit margin; the returned indices/scores are exact.

fp32r note: TRN2's full-rate fp32 matmul mode requires operands rounded
to e8m11 (top 20 bits of the fp32 encoding, RNE) - done on the host.
"""

import numpy as np

# ---- problem constants (hardcoded per contract) ----
N = 8192
M = 8192
D = 256
NCORES = 8
RPC = N // NCORES          # 1024 ref rows per core
PT = 128                   # partition tile (rows per tile)
NT = RPC // PT             # 8 row tiles per core
BLK = 512                  # column block
NJ = M // BLK              # 16 column blocks
UB_MARGIN = 6e-3           # covers fp32r rounding + exp LUT + R-hat noise
CUSHION = 5e-3             # preliminary-ranking cushion before exact R/C

_COMPILED = {}


def _round_f32r(x: np.ndarray) -> np.ndarray:
    """Round float32 to fp32r (e8m11: top 20 bits of the fp32 encoding), RNE."""
    u = np.ascontiguousarray(x, np.float32).view(np.uint32).astype(np.uint64)
    lsb = (u >> np.uint64(12)) & np.uint64(1)
    r = (u + np.uint64(0x7FF) + lsb) & np.uint64(0xFFFFF000)
    return r.astype(np.uint32).view(np.float32)


def _build_program():
    from contextlib import ExitStack

    import concourse.mybir as mybir
    import concourse.tile as tile
    from concourse import bacc

    F32 = mybir.dt.float32
    F32R = mybir.dt.float32r
    JB = 2 * BLK              # 1024-wide ScalarE tile (2 column blocks)
    NJ2 = M // JB             # 8

    nc = bacc.Bacc("TRN2", target_bir_lowering=False, debug=False,
                   num_devices=NCORES)

    refT = nc.dram_tensor("refT", [D, RPC], F32R, kind="ExternalInput").ap()
    srcT = nc.dram_tensor("srcT", [D, M], F32R, kind="ExternalInput").ap()
    nrneg = nc.dram_tensor("nrneg", [PT, NT], F32, kind="ExternalInput").ap()
    onescol = nc.dram_tensor("onescol", [PT, 1], F32R, kind="ExternalInput").ap()

    rpart = nc.dram_tensor("rpart", [PT, NT * NJ2], F32, kind="ExternalOutput").ap()
    bmout = nc.dram_tensor("bmout", [PT, NT * NJ], F32, kind="ExternalOutput").ap()
    csub = nc.dram_tensor("csub", [1, M], F32, kind="ExternalOutput").ap()

    with tile.TileContext(nc) as tc, ExitStack() as ctx:
        sb = ctx.enter_context(tc.tile_pool(name="sb", bufs=1))
        spool = ctx.enter_context(tc.tile_pool(name="spool", bufs=3))
        ps_h = ctx.enter_context(tc.tile_pool(name="ps_h", bufs=2, space="PSUM"))
        ps_c = ctx.enter_context(tc.tile_pool(name="ps_c", bufs=2, space="PSUM"))

        rT = [sb.tile([PT, RPC], F32R, tag=f"rT{h}", name=f"rT{h}")
              for h in range(2)]
        sT = [sb.tile([PT, M], F32R, tag=f"sT{h}", name=f"sT{h}")
              for h in range(2)]
        nrt = sb.tile([PT, NT], F32, tag="nrt")
        oc = sb.tile([PT, 1], F32R, tag="oc")
        rp = sb.tile([PT, NT * NJ2], F32, tag="rp")
        bm = sb.tile([PT, NT * NJ], F32, tag="bm")
        cs = sb.tile([1, M], F32, tag="cs")

        nc.sync.dma_start(out=nrt[:], in_=nrneg[:, :])
        nc.sync.dma_start(out=oc[:], in_=onescol[:, :])
        # prologue: just enough for the first tile-pair, then stream the rest
        for h in range(2):
            nc.sync.dma_start(out=rT[h][:, 0:PT], in_=refT[h * PT:(h + 1) * PT, 0:PT])
        for h in range(2):
            nc.sync.dma_start(out=sT[h][:, 0:JB], in_=srcT[h * PT:(h + 1) * PT, 0:JB])
        for h in range(2):
            nc.sync.dma_start(out=rT[h][:, PT:RPC],
                              in_=refT[h * PT:(h + 1) * PT, PT:RPC])
        for j2 in range(1, NJ2):
            cols = slice(j2 * JB, (j2 + 1) * JB)
            for h in range(2):
                nc.sync.dma_start(out=sT[h][:, cols],
                                  in_=srcT[h * PT:(h + 1) * PT, cols])

        pending = None   # (s2, cps, start, stop)
        done_cps = None  # (cps, j2) finished colsum pair needing copy-out

        for j2 in range(NJ2):
            c0 = j2 * JB
            cps = [ps_c.tile([1, BLK], F32, tag=f"cps{i}", name=f"cps{i}_{j2}")
                   for i in range(2)]
            for t in range(NT):
                rows = slice(t * PT, (t + 1) * PT)
                h2 = ps_h.tile([PT, JB], F32, tag="h2")
                for half in range(2):
                    hc = slice(c0 + half * BLK, c0 + (half + 1) * BLK)
                    po = h2[:, half * BLK:(half + 1) * BLK]
                    nc.tensor.matmul(po, rT[0][:, rows], sT[0][:, hc],
                                     start=True, stop=False)
                    nc.tensor.matmul(po, rT[1][:, rows], sT[1][:, hc],
                                     start=False, stop=True)
                if pending is not None:
                    s2_, cps_, st_, sp_ = pending
                    for half in range(2):
                        nc.tensor.matmul(
                            cps_[half][:], oc[:],
                            s2_[:, half * BLK:(half + 1) * BLK],
                            start=st_, stop=sp_)
                    pending = None
                    if sp_:
                        done_cps = (cps_, j2 - 1)
                if done_cps is not None:
                    cps_, jd = done_cps
                    for half in range(2):
                        dst = cs[:, jd * JB + half * BLK:
                                 jd * JB + (half + 1) * BLK]
                        if half == 0:
                            nc.vector.tensor_copy(dst, cps_[half][:])
                        else:
                            nc.scalar.copy(dst, cps_[half][:])
                    done_cps = None
                s2 = spool.tile([PT, JB], F32R, tag="s2")
                nc.scalar.activation(s2[:], h2[:],
                                     mybir.ActivationFunctionType.Exp,
                                     bias=nrt[:, t:t + 1], scale=1.0,
                                     accum_out=rp[:, t * NJ2 + j2:t * NJ2 + j2 + 1])
                for half in range(2):
                    kk = t * NJ + 2 * j2 + half
                    nc.vector.reduce_max(
                        bm[:, kk:kk + 1],
                        s2[:, half * BLK:(half + 1) * BLK].bitcast(F32),
                        axis=mybir.AxisListType.X)
                pending = (s2, cps, t == 0, t == NT - 1)

        s2_, cps_, st_, sp_ = pending
        for half in range(2):
            nc.tensor.matmul(cps_[half][:], oc[:],
                             s2_[:, half * BLK:(half + 1) * BLK],
                             start=st_, stop=sp_)
            dst = cs[:, (NJ2 - 1) * JB + half * BLK:
                     (NJ2 - 1) * JB + (half + 1) * BLK]
            if half == 0:
                nc.vector.tensor_copy(dst, cps_[half][:])
            else:
                nc.scalar.copy(dst, cps_[half][:])

        nc.sync.dma_start(out=rpart[:, :], in_=rp[:])
        nc.sync.dma_start(out=csub[:, :], in_=cs[:])
        nc.sync.dma_start(out=bmout[:, :], in_=bm[:])

    nc.compile()
    return nc


def _get_program():
    if "nc" not in _COMPILED:
        _COMPILED["nc"] = _build_program()
    return _COMPILED["nc"]


def _run_device(ref: np.ndarray, src: np.ndarray, perm: np.ndarray,
                nr64: np.ndarray, trace: bool = False):
    """Run the SPMD program. `perm` is the ns-ascending column permutation;
    srcT is uploaded in permuted order and all per-column outputs are in
    permuted space."""
    from concourse.bass_utils import run_bass_kernel_spmd

    nc = _get_program()

    src_p = src[perm]                                       # [M, D] permuted
    srcT_r = _round_f32r(np.ascontiguousarray(src_p.T))     # [D, M]
    onescol = np.ones((PT, 1), np.float32)

    in_maps = []
    for c in range(NCORES):
        rows = slice(c * RPC, (c + 1) * RPC)
        # factor 2 from d2 = nr + ns - 2P folded into the ref operand
        refT_r = _round_f32r(np.ascontiguousarray(2.0 * ref[rows].T))
        nrneg = np.ascontiguousarray(
            (-nr64[rows]).astype(np.float32).reshape(NT, PT).T)  # [PT, NT]
        in_maps.append({
            "refT": refT_r, "srcT": srcT_r, "nrneg": nrneg,
            "onescol": onescol,
        })

    out = run_bass_kernel_spmd(nc, in_maps, core_ids=list(range(NCORES)),
                               trace=trace)
    res = out.results

    # assemble; row n = c*RPC + t*PT + p, columns in permuted space
    NJ2 = NJ // 2
    RP = np.empty((N, NJ2), np.float64)  # 1024-wide block partials
    BM = np.empty((N, NJ), np.float64)   # block max of T (fp32r-rounded)
    B = np.zeros(M, np.float64)          # colsum of T, permuted
    for c in range(NCORES):
        rp = res[c]["rpart"].astype(np.float64).reshape(PT, NT, NJ2)
        bmv = res[c]["bmout"].astype(np.float64).reshape(PT, NT, NJ)
        rows = slice(c * RPC, (c + 1) * RPC)
        RP[rows] = rp.transpose(1, 0, 2).reshape(RPC, NJ2)
        BM[rows] = bmv.transpose(1, 0, 2).reshape(RPC, NJ)
        B += res[c]["csub"].astype(np.float64).reshape(M)
    return RP, BM, B, out


def _host_topk(ref, src, k, perm, nr64, ns64, RP, BM, B):
    """Exact top-k of F from device summaries.

    T[n,m'] = exp(2 P - nr[n]) over permuted columns m'. Device gives
    row-block partial sums RP, block maxima BM (of fp32r-rounded T) and
    column sums B. Selection uses bounds with margin; final ranking and
    scores use exact f64 R/C for candidate rows/cols only.
    """
    ref64 = ref.astype(np.float64)
    src64 = src.astype(np.float64)
    ns_p = ns64[perm]                                        # permuted ns
    es_p = np.exp(-ns_p)

    C_p = es_p * B                                           # exact-grade C
    esbar = es_p.reshape(NJ // 2, 2 * BLK).mean(axis=1)      # [NJ//2]
    Rhat = RP @ esbar                                        # [N] estimate
    logR = np.log(Rhat + 1e-8)
    logC_p = np.log(C_p + 1e-8)

    # G/2[n,m'] = 2P - nr[n] - ns[m'] - 0.5 logR[n] - 0.5 logC[m']
    #   max(2P) over block = log(BM) + nr[n]   (nr cancels in the bound)
    # UB[n,j] = log(BM[n,j]) - 0.5 logR[n] + max_{m' in j}(-ns - 0.5 logC)
    percol = -ns_p - 0.5 * logC_p                            # [M] permuted
    w = percol.reshape(NJ, BLK).max(axis=1)                  # [NJ]
    UB = np.log(np.maximum(BM, 1e-300)) - 0.5 * logR[:, None] \
        + w[None, :] + UB_MARGIN

    flat = UB.reshape(-1)
    order = np.argsort(-flat)

    cand_val = []   # G/2 using device-derived R-hat/C (selection-grade)
    cand_idx = []   # flat n*M + m (TRUE column ids)
    kth_best = -np.inf
    pos = 0
    CHUNK = 64
    total = flat.shape[0]
    while pos < total:
        if len(cand_val) * BLK >= k and kth_best >= flat[order[pos]]:
            break
        take = order[pos:pos + CHUNK]
        pos += CHUNK
        for b in take:
            n = int(b // NJ)
            j = int(b % NJ)
            mcols = perm[j * BLK:(j + 1) * BLK]              # true col ids
            p = src64[mcols] @ ref64[n]                      # [BLK]
            g2 = (2.0 * p - nr64[n] - ns64[mcols]
                  - 0.5 * logR[n] - 0.5 * logC_p[j * BLK:(j + 1) * BLK])
            cand_val.append(g2)
            cand_idx.append(n * M + mcols.astype(np.int64))
        vals = np.concatenate(cand_val)
        if vals.shape[0] >= k:
            kth_best = np.partition(vals, -k)[-k]

    vals = np.concatenate(cand_val)
    idxs = np.concatenate(cand_idx)

    kth = np.partition(vals, -k)[-k]
    keep = vals >= kth - CUSHION
    cidx = idxs[keep]
    rows = cidx // M
    cols = cidx % M

    # exact R/C (f64 from original f32 inputs) for candidate rows/cols
    urows = np.unique(rows)
    ucols = np.unique(cols)
    Pr = ref64[urows] @ src64.T
    Rex_map = np.exp(-(nr64[urows][:, None] + ns64[None, :] - 2.0 * Pr)).sum(1)
    Pc = src64[ucols] @ ref64.T
    Cex_map = np.exp(-(ns64[ucols][:, None] + nr64[None, :] - 2.0 * Pc)).sum(1)
    Rex = np.empty(N)
    Rex[urows] = Rex_map
    Cex = np.empty(M)
    Cex[ucols] = Cex_map

    d2 = nr64[rows] + ns64[cols] - 2.0 * np.einsum(
        "ij,ij->i", ref64[rows], src64[cols])
    S = np.exp(-d2)
    F = (S / (Rex[rows] + 1e-8)) * (S / (Cex[cols] + 1e-8))

    top = np.argpartition(F, -k)[-k:]
    tidx = cidx[top]
    F32v = F[top].astype(np.float32)
    # mirror lax.top_k ordering: f32 value desc, flat index asc on ties
    ordr = np.lexsort((tidx, -F32v.astype(np.float64)))
    return tidx[ordr], F32v[ordr]


def kernel(ref_feats, src_feats, num_correspondences):
    k = int(np.asarray(num_correspondences))
    ref = np.ascontiguousarray(np.asarray(ref_feats), dtype=np.float32)
    src = np.ascontiguousarray(np.asarray(src_feats), dtype=np.float32)
    assert ref.shape == (N, D) and src.shape == (M, D)

    nr64 = (ref.astype(np.float64) ** 2).sum(1)
    ns64 = (src.astype(np.float64) ** 2).sum(1)
    perm = np.argsort(ns64, kind="stable")

    RP, BM, B, _ = _run_device(ref, src, perm, nr64)
    tidx, F32v = _host_topk(ref, src, k, perm, nr64, ns64, RP, BM, B)

    ref_idx = (tidx // M).astype(np.int32)
    src_idx = (tidx % M).astype(np.int32)
    return ref_idx, src_idx, F32v.astype(np.float32)
